# revision 54
# baseline (speedup 1.0000x reference)
"""GRUCell + LayerNorm readout fused Bass kernel for Trainium2 (8 NeuronCores).

Problem: B=8192, D=H=O=1024 fp32.
    r = sigmoid(x@Wir + bir + h@Whr)
    z = sigmoid(x@Wiz + biz + h@Whz)
    n = tanh(x@Win + bin_ + r*(h@Whn + bhn))
    new_h = (1-z)*n + z*h
    out = (LayerNorm(new_h)*ln_scale + ln_bias) @ Wout + bout

Strategy:
  - Data-parallel over batch: core c gets rows [c*1024, (c+1)*1024); weights
    replicated. No collectives.
  - Transposed domain: activations live as [feature, batch]; per-h gate
    biases become per-partition activation biases.
  - All big matmuls in bf16 (host pre-casts + pre-swizzles weights/inputs to
    the exact SBUF layout, so every DMA is 2KB-per-partition contiguous).
    Measured end-to-end error ~5.4e-3 vs the 2e-2 gate. Stats/broadcast
    matmuls stay f32r (exact fp32 width). fp8 was evaluated and rejected:
    e4m3 r/z gates alone measure 4.4e-2.
  - PSUM A/B alternation: each (ht, bc) group of 48 matmuls uses 4 banks
    (r,z,gi,gh) of one parity; its epilogue chain drains under the next
    group's 10.4us matmul stream, so the PE never waits on PSUM reuse.
  - PE warmup: 12 dummy matmuls at t=0 cover the input-DMA ramp and lift
    the HAM clock gate (1.2->2.4GHz) before real work arrives.
  - Ramp DMA: dma_start triggers cost ~0.66us of queue occupancy each, so
    the ramp-critical loads go on ONE queue (sync) in exact PE consumption
    order (x0, Wir0..Win0, x1..x3, Whr0..Whn0, h0.., x/h interleave).
    Steady-state weight tiles double-buffer one ht ahead. gpsimd queue
    carries woutF half-blocks + nhT stores; outT stores ride sync (idle in
    phase 2).
  - LayerNorm folded into the readout:
        LN(new_h) @ (ln_scale*Wout) + (ln_bias@Wout + bout)
      = rstd[b]*new_h@WoutF + m2[b]*colsum[o] + boutF[o],  m2 = -mu*rstd
    rstd/m2 row vectors are broadcast across partitions by ones-row
    matmuls, so each readout group's finalize is 3 element-wise ops spread
    over GpSimd+DVE with zero PE involvement, and the accumulation group is
    exactly the 8 contraction matmuls. LN stats accumulate elementwise over
    h-tiles, reduce across partitions via a ones-column matmul, and the
    1/sqrt uses reciprocal_approx_fast (the exact DVE reciprocal is 3.9us
    for 512 lanes on one partition and was the phase-2 critical path); the
    whole stats chain for bc1 is emitted mid-final-group so it hides under
    the last 24 gate matmuls + first readout groups (phase 2 runs bc1
    first, and the final phase-1 group order is swapped to bc1, bc0).
"""

import sys
from contextlib import ExitStack

sys.path.insert(0, "/opt/trn_rl_repo")

import numpy as np
import ml_dtypes

import concourse.bacc as bacc
import concourse.mybir as mybir
import concourse.tile as tile
from concourse import bass_utils

B, D, H, O = 8192, 1024, 1024, 1024
NCORES = 8
BL = B // NCORES          # batch rows per core
P = 128                   # partitions
KT = D // P               # contraction tiles (8)
HT = H // P               # h output-partition tiles (8)
OT = O // P               # o output-partition tiles (8)
NB = 2                    # batch chunks per core
NF = BL // NB             # free dim per chunk (512)
LN_EPS = 1e-6
NWARM = 12                # PE warmup matmuls

F32 = mybir.dt.float32
F32R = mybir.dt.float32r
BF16 = mybir.dt.bfloat16
NPBF16 = ml_dtypes.bfloat16

_COMPILED = None
TRACE = False
LAST_RES = None

GATES = ("ir", "iz", "in", "hr", "hz", "hn")


def _build():
    nc = bacc.Bacc("TRN2", target_bir_lowering=False, debug=False,
                   num_devices=NCORES)

    # all per-partition-contiguous layouts so DMA descriptors are 2-16KB
    xT_d = nc.dram_tensor("xT", [P, KT, BL], BF16, kind="ExternalInput").ap()
    hT_d = nc.dram_tensor("hT", [P, KT, BL], BF16, kind="ExternalInput").ap()
    wall_d = nc.dram_tensor("wall", [HT, P, len(GATES), KT, P], BF16,
                            kind="ExternalInput").ap()
    wout_d = nc.dram_tensor("woutF", [P, OT, KT, P], BF16,
                            kind="ExternalInput").ap()
    # [P, 50]: bir | biz | bin | bhn | boutF (8 cols each, col t = tile t),
    # col 40 = ones (stats-reduce stationary), cols 41:49 = colsum tiles
    bvec_d = nc.dram_tensor("bvec", [P, 50], F32R, kind="ExternalInput").ap()
    # [1, P]: ones_row (broadcast stationary)
    rowv_d = nc.dram_tensor("rowv", [1, P], F32R,
                            kind="ExternalInput").ap()

    nhT_d = nc.dram_tensor("nhT", [HT, P, BL], BF16,
                           kind="ExternalOutput").ap()
    outT_d = nc.dram_tensor("outT", [OT, P, BL], BF16,
                            kind="ExternalOutput").ap()

    with tile.TileContext(nc) as tc, ExitStack() as ctx:
        singles = ctx.enter_context(tc.tile_pool(name="singles", bufs=1))
        wpool = ctx.enter_context(tc.tile_pool(name="wpool", bufs=2))
        gates = ctx.enter_context(tc.tile_pool(name="gates", bufs=1))
        rows = ctx.enter_context(tc.tile_pool(name="rows", bufs=1))
        ps = ctx.enter_context(tc.tile_pool(name="ps", bufs=1, space="PSUM"))

        TAGS = [["a0", "b0", "c0", "d0"], ["a1", "b1", "c1", "d1"]]

        # ---- PE warmup: dummy matmuls cover the DMA ramp, lift HAM -------
        wm = singles.tile([P, 2 * P], BF16, tag="wm", name="wm")
        nc.vector.memset(wm[:], 0.0)
        for i in range(NWARM):
            pw = ps.tile([P, P], F32, tag="a1", name=f"warm{i}")
            nc.tensor.matmul(pw[:], wm[:, 0:P], wm[:, P:2 * P],
                             start=True, stop=True)

        # ---- resident inputs --------------------------------------------
        x_sb = singles.tile([P, KT, BL], BF16, tag="x_sb", name="x_sb")
        h_sb = singles.tile([P, KT, BL], BF16, tag="h_sb", name="h_sb")

        def wtile(g, ht):
            t = wpool.tile([P, KT, P], BF16, tag=f"w{g}", name=f"w{g}_{ht}")
            nc.sync.dma_start(t[:], wall_d[ht][:, GATES.index(g), :, :])
            return t

        def load_wht(ht):
            return {g: wtile(g, ht) for g in GATES}

        def wsl(w, g, k):
            return w[g][:, k, :]

        # ramp: ONE queue (per-queue FIFO) in exact PE consumption order,
        # per-k slices so arrival granularity matches consumption
        w_cur = {}
        nc.sync.dma_start(x_sb[:, 0, :], xT_d[:, 0, :])
        for g in ("ir", "iz", "in"):
            w_cur[g] = wtile(g, 0)
        for k in (1, 2, 3):
            nc.sync.dma_start(x_sb[:, k, :], xT_d[:, k, :])
        for g in ("hr", "hz", "hn"):
            w_cur[g] = wtile(g, 0)
        for k in (0, 1):
            nc.sync.dma_start(h_sb[:, k, :], hT_d[:, k, :])
        for xk, hk in ((4, 2), (5, 3), (6, 4), (7, 5)):
            nc.sync.dma_start(x_sb[:, xk, :], xT_d[:, xk, :])
            nc.sync.dma_start(h_sb[:, hk, :], hT_d[:, hk, :])
        for k in (6, 7):
            nc.sync.dma_start(h_sb[:, k, :], hT_d[:, k, :])
        w_nxt = load_wht(1)

        bvec = singles.tile([P, 50], F32R, tag="bvec", name="bvec")
        nc.scalar.dma_start(bvec[:], bvec_d)
        rowv = singles.tile([1, P], F32R, tag="rowv", name="rowv")
        nc.scalar.dma_start(rowv[:], rowv_d)
        eps_sb = singles.tile([1, 1], F32, tag="eps", name="eps")
        nc.vector.memset(eps_sb[:], LN_EPS)

        def bias_col(v, t):  # v: 0=bir 1=biz 2=bin 3=bhn 4=boutF
            return bvec[:, 8 * v + t:8 * v + t + 1].bitcast(F32)

        ones_col = bvec[:, 40:41]
        ones_row = rowv[:, 0:P]

        def colsum_col(ot):  # [P,1] per-partition scalar, f32 view
            return bvec[:, 41 + ot:42 + ot].bitcast(F32)

        wout_sb = singles.tile([P, OT, KT, P], BF16, tag="wout",
                               name="wout_sb")

        nh_sb = [singles.tile([P, BL], BF16, tag=f"nh{ht}", name=f"nh{ht}")
                 for ht in range(HT)]
        s_acc = [singles.tile([P, NF], F32R, tag=f"s_acc{bc}",
                              name=f"s_acc{bc}") for bc in range(NB)]
        q_acc = [singles.tile([P, NF], F32R, tag=f"q_acc{bc}",
                              name=f"q_acc{bc}") for bc in range(NB)]

        # ---- LN stats helpers (emitted inside/right after phase 1 so the
        # ---- 3.9us DVE RECIPROCAL hides under matmul cover) ---------------
        rstd_row = {}
        m2_row = {}
        rstd_bc = {}
        m2_bc = {}

        nmu_row = {}
        var_row = {}

        def emit_stats_pre(bc):
            # PE reduce matmuls + DVE-only var chain (no ACT involvement,
            # so the group-15 sigmoid/tanh sequence never waits on an ACT
            # table reload for Sqrt)
            psum_s = ps.tile([1, NF], F32, tag="c0", name=f"psum_s{bc}")
            nc.tensor.matmul(psum_s[:], ones_col, s_acc[bc][:],
                             start=True, stop=True)
            psum_q = ps.tile([1, NF], F32, tag="d0", name=f"psum_q{bc}")
            nc.tensor.matmul(psum_q[:], ones_col, q_acc[bc][:],
                             start=True, stop=True)

            nmu = rows.tile([1, NF], F32, tag=f"nmu{bc}", name=f"nmu{bc}")
            nc.vector.tensor_scalar_mul(nmu[:], psum_s[:], -1.0 / H)
            nmu_row[bc] = nmu

            mu2 = rows.tile([1, NF], F32, tag="mu2", name=f"mu2_{bc}")
            nc.vector.tensor_mul(mu2[:], nmu[:], nmu[:])
            var = rows.tile([1, NF], F32, tag=f"var{bc}", name=f"var_{bc}")
            nc.vector.tensor_scalar_mul(var[:], psum_q[:], 1.0 / H)
            nc.vector.tensor_tensor(var[:], var[:], mu2[:],
                                    mybir.AluOpType.subtract)
            var_row[bc] = var

        def emit_stats_post(bc):
            # ACT sqrt (emitted after all phase-1 activations -> single
            # table load), then the DVE reciprocal chain
            var = var_row[bc]
            nc.scalar.activation(var[:], var[:],
                                 mybir.ActivationFunctionType.Sqrt,
                                 bias=eps_sb[:])
            rec = rows.tile([1, NF], F32, tag=f"rec{bc}", name=f"rec{bc}")
            nc.vector.reciprocal_approx_fast(rec[:], var[:])
            rrow = rows.tile([1, NF], F32R, tag=f"rstd{bc}",
                             name=f"rstd{bc}")
            nc.vector.tensor_copy(rrow[:], rec[:])
            rstd_row[bc] = rrow
            m2 = rows.tile([1, NF], F32R, tag=f"m2_{bc}", name=f"m2_{bc}")
            nc.vector.tensor_tensor(m2[:], rec[:], nmu_row[bc][:],
                                    mybir.AluOpType.mult)
            m2_row[bc] = m2

        def emit_bcast(bc):
            pb = ps.tile([P, NF], F32, tag="c0", name=f"pb{bc}")
            nc.tensor.matmul(pb[:], ones_row, rstd_row[bc][:],
                             start=True, stop=True)
            rb = rows.tile([P, NF], F32, tag=f"rstd_bc{bc}",
                           name=f"rstd_bc{bc}")
            nc.vector.tensor_copy(rb[:], pb[:])
            rstd_bc[bc] = rb
            pm = ps.tile([P, NF], F32, tag="d0", name=f"pm{bc}")
            nc.tensor.matmul(pm[:], ones_row, m2_row[bc][:],
                             start=True, stop=True)
            mb = rows.tile([P, NF], F32, tag=f"m2_bc{bc}",
                           name=f"m2_bc{bc}")
            nc.vector.tensor_copy(mb[:], pm[:])
            m2_bc[bc] = mb

        # ---- phase 1: gates + new_h --------------------------------------
        gidx = 0  # global group counter -> PSUM parity
        for ht in range(HT):
            # last ht runs bc1 first so s_acc[1] finalizes one group early
            # (its stats chain then hides under the final group + po stream)
            for bc in (range(NB) if ht < HT - 1 else (1, 0)):
                s = gidx % 2
                bs = slice(bc * NF, (bc + 1) * NF)
                pr = ps.tile([P, NF], F32, tag=TAGS[s][0],
                             name=f"pr{ht}_{bc}")
                pz = ps.tile([P, NF], F32, tag=TAGS[s][1],
                             name=f"pz{ht}_{bc}")
                pgi = ps.tile([P, NF], F32, tag=TAGS[s][2],
                              name=f"pgi{ht}_{bc}")
                pgh = ps.tile([P, NF], F32, tag=TAGS[s][3],
                              name=f"pgh{ht}_{bc}")

                for k in range(KT):
                    xs = x_sb[:, k, bs]
                    nc.tensor.matmul(pr[:], wsl(w_cur, "ir", k), xs,
                                     start=(k == 0), stop=False)
                    nc.tensor.matmul(pz[:], wsl(w_cur, "iz", k), xs,
                                     start=(k == 0), stop=False)
                    nc.tensor.matmul(pgi[:], wsl(w_cur, "in", k), xs,
                                     start=(k == 0), stop=(k == KT - 1))
                if ht == HT - 1 and bc == 0:
                    # stats(1) reduce+var mid-final-group: hides under the
                    # h-side + po stream (phase 2 runs bc1 first)
                    emit_stats_pre(1)
                for k in range(KT):
                    hs = h_sb[:, k, bs]
                    nc.tensor.matmul(pr[:], wsl(w_cur, "hr", k), hs,
                                     start=False, stop=(k == KT - 1))
                    nc.tensor.matmul(pz[:], wsl(w_cur, "hz", k), hs,
                                     start=False, stop=(k == KT - 1))
                    nc.tensor.matmul(pgh[:], wsl(w_cur, "hn", k), hs,
                                     start=(k == 0), stop=(k == KT - 1))

                # epilogue: drains under the next group's matmul stream
                r_sb = gates.tile([P, NF], F32, tag="r_act", name="r_sb")
                nc.scalar.activation(r_sb[:], pr[:],
                                     mybir.ActivationFunctionType.Sigmoid,
                                     bias=bias_col(0, ht))
                z_sb = gates.tile([P, NF], F32, tag="z_act", name="z_sb")
                nc.scalar.activation(z_sb[:], pz[:],
                                     mybir.ActivationFunctionType.Sigmoid,
                                     bias=bias_col(1, ht))

                t_sb = gates.tile([P, NF], F32, tag="t", name="t_sb")
                nc.vector.tensor_scalar(t_sb[:], pgh[:], bias_col(3, ht),
                                        None, mybir.AluOpType.add)
                nc.vector.tensor_mul(t_sb[:], t_sb[:], r_sb[:])
                nc.vector.tensor_add(t_sb[:], t_sb[:], pgi[:])
                n_sb = gates.tile([P, NF], F32, tag="n", name="n_sb")
                nc.scalar.activation(n_sb[:], t_sb[:],
                                     mybir.ActivationFunctionType.Tanh,
                                     bias=bias_col(2, ht))

                u_sb = gates.tile([P, NF], F32, tag="u", name="u_sb")
                nc.vector.tensor_tensor(u_sb[:], h_sb[:, ht, bs], n_sb[:],
                                        mybir.AluOpType.subtract)
                nc.vector.tensor_mul(u_sb[:], z_sb[:], u_sb[:])
                nhv = nh_sb[ht][:, bs]
                nc.vector.tensor_add(nhv, n_sb[:], u_sb[:])

                # LN stat partials (elementwise over h-tiles)
                if ht == 0:
                    nc.vector.tensor_copy(s_acc[bc][:], nhv)
                    nc.scalar.activation(q_acc[bc][:], nhv,
                                         mybir.ActivationFunctionType.Square)
                else:
                    sq_sb = gates.tile([P, NF], F32, tag="sq", name="sq_sb")
                    nc.vector.tensor_tensor(s_acc[bc][:],
                                            s_acc[bc][:].bitcast(F32), nhv,
                                            mybir.AluOpType.add)
                    nc.scalar.activation(sq_sb[:], nhv,
                                         mybir.ActivationFunctionType.Square)
                    nc.vector.tensor_tensor(q_acc[bc][:],
                                            q_acc[bc][:].bitcast(F32),
                                            sq_sb[:],
                                            mybir.AluOpType.add)
                gidx += 1

            nc.gpsimd.dma_start(nhT_d[ht], nh_sb[ht][:])
            if ht + 2 < HT:
                w_cur = w_nxt
                w_nxt = load_wht(ht + 2)
            elif ht + 2 == HT:
                w_cur = w_nxt
            # readout weights in two half-blocks mid-phase-1
            if ht == 2:
                nc.gpsimd.dma_start(wout_sb[:, 0:4, :, :],
                                    wout_d[:, 0:4, :, :])
            elif ht == 4:
                nc.gpsimd.dma_start(wout_sb[:, 4:OT, :, :],
                                    wout_d[:, 4:OT, :, :])

        # ---- phase 2: readout ---------------------------------------------
        # out = rstd*po - (rstd*mu)*colsum + boutF; rstd and m2=rstd*(-mu)
        # are row vectors broadcast across partitions by a ones-row matmul,
        # so the per-group finalize has no PE involvement (t1 on GpSimd, the
        # two [P,NF] combines on DVE).
        po_tags = ["a0", "b0", "a1", "b1", "c1", "d1"]
        groups = [(ot, 1) for ot in range(OT)] + [(ot, 0) for ot in range(OT)]

        t1_tiles = {}

        def emit_t1s(bc):
            # t1 = m2*colsum + boutF depends only on m2_bc: emit the whole
            # bc-batch at once so GpSimd drains early, off the tail path
            for ot in range(OT):
                t = gates.tile([P, NF], F32, tag=f"t1_{ot}",
                               name=f"t1_{ot}_{bc}")
                nc.gpsimd.tensor_scalar(t[:], m2_bc[bc][:], colsum_col(ot),
                                        bias_col(4, ot),
                                        mybir.AluOpType.mult,
                                        mybir.AluOpType.add)
                t1_tiles[(ot, bc)] = t

        def finalize(i, ot, bc, po):
            # PE-free: two DVE combines + store
            o_sb = gates.tile([P, NF], F32, tag=("r_act", "z_act", "n")[i % 3],
                              name=f"o_{ot}_{bc}")
            nc.vector.tensor_mul(o_sb[:], po[:], rstd_bc[bc][:])
            ob = gates.tile([P, NF], BF16, tag=("w1", "w2", "w3")[i % 3],
                            name=f"ob_{ot}_{bc}")
            # final add on GpSimd (idle once the t1s are done): halves the
            # DVE per-group load so finalizes don't pile up past the last
            # matmul
            nc.gpsimd.tensor_tensor(ob[:], o_sb[:], t1_tiles[(ot, bc)][:],
                                    mybir.AluOpType.add)
            nc.sync.dma_start(outT_d[ot][:, bs_of(bc)], ob[:])

        def bs_of(bc):
            return slice(bc * NF, (bc + 1) * NF)

        emit_stats_post(1)
        pending = []
        for i, (ot, bc) in enumerate(groups):
            po = ps.tile([P, NF], F32, tag=po_tags[i % len(po_tags)],
                         name=f"po_{ot}_{bc}")
            for k in range(HT):
                nc.tensor.matmul(po[:], wout_sb[:, ot, k, :],
                                 nh_sb[k][:, bs_of(bc)],
                                 start=(k == 0), stop=(k == HT - 1))
            if i == 2:
                emit_stats_pre(0)
            elif i == 3:
                # bcast(1) + the bc0 sqrt chain, then flush the finalizes
                # that had to wait for rstd_bc[1]/m2_bc[1] to be enqueued
                emit_bcast(1)
                emit_stats_post(0)
                emit_t1s(1)
            elif i == 5:
                emit_bcast(0)
                emit_t1s(0)
            if i < 3:
                pending.append((i, ot, bc, po))
            else:
                for args in pending:
                    finalize(*args)
                pending = []
                finalize(i, ot, bc, po)

    nc.compile()
    return nc


def _swz_w(w):
    # [D, H] -> [HT, P, KT, P] with out[ht, p, t, c] = w[t*P+p, ht*P+c]
    return np.ascontiguousarray(
        w.reshape(KT, P, HT, P).transpose(2, 1, 0, 3)).astype(NPBF16)


def kernel(x, h, Wir, bir, Wiz, biz, Win, bin_, Whr, Whz, Whn, bhn,
           ln_scale, ln_bias, Wout, bout):
    global _COMPILED, LAST_RES
    if _COMPILED is None:
        _COMPILED = _build()
    nc = _COMPILED

    f = lambda a: np.asarray(a, np.float32)
    x, h = f(x), f(h)
    Wout, ln_scale, ln_bias = f(Wout), f(ln_scale), f(ln_bias)

    woutF = ln_scale[:, None] * Wout
    boutF = f(bout) + ln_bias @ Wout
    colsum = ln_scale @ Wout

    wall = np.empty((HT, P, len(GATES), KT, P), dtype=NPBF16)
    for gi, w in enumerate((Wir, Wiz, Win, Whr, Whz, Whn)):
        wall[:, :, gi] = _swz_w(f(w))
    wout_swz = np.ascontiguousarray(_swz_w(woutF).transpose(1, 0, 2, 3))

    bvec = np.zeros((P, 50), np.float32)
    for vi, v in enumerate((bir, biz, bin_, bhn, boutF)):
        bvec[:, 8 * vi:8 * (vi + 1)] = f(v).reshape(HT, P).T
    bvec[:, 40] = 1.0
    bvec[:, 41:49] = colsum.reshape(OT, P).T
    rowv = np.ones((1, P), np.float32)

    common = {"wall": wall, "woutF": wout_swz, "bvec": bvec, "rowv": rowv}
    in_maps = []
    for c in range(NCORES):
        bsl = slice(c * BL, (c + 1) * BL)
        xT = x[bsl].T.reshape(KT, P, BL).transpose(1, 0, 2)
        hT = h[bsl].T.reshape(KT, P, BL).transpose(1, 0, 2)
        in_maps.append({
            **common,
            "xT": np.ascontiguousarray(xT).astype(NPBF16),
            "hT": np.ascontiguousarray(hT).astype(NPBF16),
        })

    res = bass_utils.run_bass_kernel_spmd(nc, in_maps,
                                          core_ids=list(range(NCORES)),
                                          trace=TRACE)
    LAST_RES = res
    nh_parts, out_parts = [], []
    for c in range(NCORES):
        nhT = res.results[c]["nhT"].reshape(H, BL)
        outT = res.results[c]["outT"].reshape(O, BL)
        nh_parts.append(np.asarray(nhT, np.float32).T)
        out_parts.append(np.asarray(outT, np.float32).T)
    new_h = np.ascontiguousarray(np.concatenate(nh_parts, axis=0))
    out = np.ascontiguousarray(np.concatenate(out_parts, axis=0))
    return new_h, out


# revision 55
# speedup vs baseline: 1.0365x; 1.0365x over previous
"""GRUCell + LayerNorm readout fused Bass kernel for Trainium2 (8 NeuronCores).

Problem: B=8192, D=H=O=1024 fp32.
    r = sigmoid(x@Wir + bir + h@Whr)
    z = sigmoid(x@Wiz + biz + h@Whz)
    n = tanh(x@Win + bin_ + r*(h@Whn + bhn))
    new_h = (1-z)*n + z*h
    out = (LayerNorm(new_h)*ln_scale + ln_bias) @ Wout + bout

Strategy:
  - Data-parallel over batch: core c gets rows [c*1024, (c+1)*1024); weights
    replicated. No collectives.
  - Transposed domain: activations live as [feature, batch]; per-h gate
    biases become per-partition activation biases.
  - All big matmuls in bf16 (host pre-casts + pre-swizzles weights/inputs to
    the exact SBUF layout, so every DMA is 2KB-per-partition contiguous).
    Measured end-to-end error ~5.4e-3 vs the 2e-2 gate. Stats/broadcast
    matmuls stay f32r (exact fp32 width). fp8 was evaluated and rejected:
    e4m3 r/z gates alone measure 4.4e-2.
  - PSUM A/B alternation: each (ht, bc) group of 48 matmuls uses 4 banks
    (r,z,gi,gh) of one parity; its epilogue chain drains under the next
    group's 10.4us matmul stream, so the PE never waits on PSUM reuse.
  - PE warmup: 12 dummy matmuls at t=0 cover the input-DMA ramp and lift
    the HAM clock gate (1.2->2.4GHz) before real work arrives.
  - Ramp DMA: dma_start triggers cost ~0.66us of queue occupancy each, so
    the ramp-critical loads go on ONE queue (sync) in exact PE consumption
    order (x0, Wir0..Win0, x1..x3, Whr0..Whn0, h0.., x/h interleave).
    Steady-state weight tiles double-buffer one ht ahead. gpsimd queue
    carries woutF half-blocks + nhT stores; outT stores ride sync (idle in
    phase 2).
  - LayerNorm folded into the readout:
        LN(new_h) @ (ln_scale*Wout) + (ln_bias@Wout + bout)
      = rstd[b]*new_h@WoutF + m2[b]*colsum[o] + boutF[o],  m2 = -mu*rstd
    rstd/m2 row vectors are broadcast across partitions by ones-row
    matmuls, so each readout group's finalize is 3 element-wise ops spread
    over GpSimd+DVE with zero PE involvement, and the accumulation group is
    exactly the 8 contraction matmuls. LN stats accumulate elementwise over
    h-tiles, reduce across partitions via a ones-column matmul, and the
    1/sqrt uses reciprocal_approx_fast (the exact DVE reciprocal is 3.9us
    for 512 lanes on one partition and was the phase-2 critical path); the
    whole stats chain for bc1 is emitted mid-final-group so it hides under
    the last 24 gate matmuls + first readout groups (phase 2 runs bc1
    first, and the final phase-1 group order is swapped to bc1, bc0).
"""

import sys
from contextlib import ExitStack

sys.path.insert(0, "/opt/trn_rl_repo")

import numpy as np
import ml_dtypes

import concourse.bacc as bacc
import concourse.mybir as mybir
import concourse.tile as tile
from concourse import bass_utils

B, D, H, O = 8192, 1024, 1024, 1024
NCORES = 8
BL = B // NCORES          # batch rows per core
P = 128                   # partitions
KT = D // P               # contraction tiles (8)
HT = H // P               # h output-partition tiles (8)
OT = O // P               # o output-partition tiles (8)
NB = 2                    # batch chunks per core
NF = BL // NB             # free dim per chunk (512)
LN_EPS = 1e-6
NWARM = 12                # PE warmup matmuls

F32 = mybir.dt.float32
F32R = mybir.dt.float32r
BF16 = mybir.dt.bfloat16
NPBF16 = ml_dtypes.bfloat16

_COMPILED = None
TRACE = False
LAST_RES = None

GATES = ("ir", "iz", "in", "hr", "hz", "hn")


def _build():
    nc = bacc.Bacc("TRN2", target_bir_lowering=False, debug=False,
                   num_devices=NCORES)

    # all per-partition-contiguous layouts so DMA descriptors are 2-16KB
    xT_d = nc.dram_tensor("xT", [P, KT, BL], BF16, kind="ExternalInput").ap()
    hT_d = nc.dram_tensor("hT", [P, KT, BL], BF16, kind="ExternalInput").ap()
    wall_d = nc.dram_tensor("wall", [HT, P, len(GATES), KT, P], BF16,
                            kind="ExternalInput").ap()
    wout_d = nc.dram_tensor("woutF", [P, OT, KT, P], BF16,
                            kind="ExternalInput").ap()
    # [P, 50]: bir | biz | bin | bhn | boutF (8 cols each, col t = tile t),
    # col 40 = ones (stats-reduce stationary), cols 41:49 = colsum tiles
    bvec_d = nc.dram_tensor("bvec", [P, 50], F32R, kind="ExternalInput").ap()
    # [1, P]: ones_row (broadcast stationary)
    rowv_d = nc.dram_tensor("rowv", [1, P], F32R,
                            kind="ExternalInput").ap()

    nhT_d = nc.dram_tensor("nhT", [HT, P, BL], BF16,
                           kind="ExternalOutput").ap()
    outT_d = nc.dram_tensor("outT", [OT, P, BL], BF16,
                            kind="ExternalOutput").ap()

    with tile.TileContext(nc) as tc, ExitStack() as ctx:
        singles = ctx.enter_context(tc.tile_pool(name="singles", bufs=1))
        wpool = ctx.enter_context(tc.tile_pool(name="wpool", bufs=2))
        gates = ctx.enter_context(tc.tile_pool(name="gates", bufs=1))
        rows = ctx.enter_context(tc.tile_pool(name="rows", bufs=1))
        ps = ctx.enter_context(tc.tile_pool(name="ps", bufs=1, space="PSUM"))

        TAGS = [["a0", "b0", "c0", "d0"], ["a1", "b1", "c1", "d1"]]

        # ---- PE warmup: dummy matmuls cover the DMA ramp, lift HAM -------
        wm = singles.tile([P, 2 * P], BF16, tag="wm", name="wm")
        nc.vector.memset(wm[:], 0.0)
        for i in range(NWARM):
            pw = ps.tile([P, P], F32, tag="a1", name=f"warm{i}")
            nc.tensor.matmul(pw[:], wm[:, 0:P], wm[:, P:2 * P],
                             start=True, stop=True)

        # ---- resident inputs --------------------------------------------
        x_sb = singles.tile([P, KT, BL], BF16, tag="x_sb", name="x_sb")
        h_sb = singles.tile([P, KT, BL], BF16, tag="h_sb", name="h_sb")

        def wtile(g, ht):
            t = wpool.tile([P, KT, P], BF16, tag=f"w{g}", name=f"w{g}_{ht}")
            nc.sync.dma_start(t[:], wall_d[ht][:, GATES.index(g), :, :])
            return t

        def load_wht(ht):
            return {g: wtile(g, ht) for g in GATES}

        def wsl(w, g, k):
            return w[g][:, k, :]

        # ramp: ONE queue (per-queue FIFO) in exact PE consumption order,
        # per-k slices so arrival granularity matches consumption
        w_cur = {}
        nc.sync.dma_start(x_sb[:, 0, :], xT_d[:, 0, :])
        for g in ("ir", "iz", "in"):
            w_cur[g] = wtile(g, 0)
        for k in (1, 2, 3):
            nc.sync.dma_start(x_sb[:, k, :], xT_d[:, k, :])
        for g in ("hr", "hz", "hn"):
            w_cur[g] = wtile(g, 0)
        for k in (0, 1):
            nc.sync.dma_start(h_sb[:, k, :], hT_d[:, k, :])
        for xk, hk in ((4, 2), (5, 3), (6, 4), (7, 5)):
            nc.sync.dma_start(x_sb[:, xk, :], xT_d[:, xk, :])
            nc.sync.dma_start(h_sb[:, hk, :], hT_d[:, hk, :])
        for k in (6, 7):
            nc.sync.dma_start(h_sb[:, k, :], hT_d[:, k, :])
        w_nxt = load_wht(1)

        bvec = singles.tile([P, 50], F32R, tag="bvec", name="bvec")
        nc.scalar.dma_start(bvec[:], bvec_d)
        rowv = singles.tile([1, P], F32R, tag="rowv", name="rowv")
        nc.scalar.dma_start(rowv[:], rowv_d)
        eps_sb = singles.tile([1, 1], F32, tag="eps", name="eps")
        nc.vector.memset(eps_sb[:], LN_EPS)

        def bias_col(v, t):  # v: 0=bir 1=biz 2=bin 3=bhn 4=boutF
            return bvec[:, 8 * v + t:8 * v + t + 1].bitcast(F32)

        ones_col = bvec[:, 40:41]
        ones_row = rowv[:, 0:P]

        def colsum_col(ot):  # [P,1] per-partition scalar, f32 view
            return bvec[:, 41 + ot:42 + ot].bitcast(F32)

        wout_sb = singles.tile([P, OT, KT, P], BF16, tag="wout",
                               name="wout_sb")

        nh_sb = [singles.tile([P, BL], BF16, tag=f"nh{ht}", name=f"nh{ht}")
                 for ht in range(HT)]
        s_acc = [singles.tile([P, NF], F32R, tag=f"s_acc{bc}",
                              name=f"s_acc{bc}") for bc in range(NB)]
        q_acc = [singles.tile([P, NF], F32R, tag=f"q_acc{bc}",
                              name=f"q_acc{bc}") for bc in range(NB)]

        # ---- LN stats helpers (emitted inside/right after phase 1 so the
        # ---- 3.9us DVE RECIPROCAL hides under matmul cover) ---------------
        rstd_row = {}
        m2_row = {}
        rstd_bc = {}
        m2_bc = {}

        nmu_row = {}
        var_row = {}

        def emit_stats_pre(bc):
            # PE reduce matmuls + DVE-only var chain (no ACT involvement,
            # so the group-15 sigmoid/tanh sequence never waits on an ACT
            # table reload for Sqrt)
            psum_s = ps.tile([1, NF], F32, tag="c0", name=f"psum_s{bc}")
            nc.tensor.matmul(psum_s[:], ones_col, s_acc[bc][:],
                             start=True, stop=True)
            psum_q = ps.tile([1, NF], F32, tag="d0", name=f"psum_q{bc}")
            nc.tensor.matmul(psum_q[:], ones_col, q_acc[bc][:],
                             start=True, stop=True)

            nmu = rows.tile([1, NF], F32, tag=f"nmu{bc}", name=f"nmu{bc}")
            nc.vector.tensor_scalar_mul(nmu[:], psum_s[:], -1.0 / H)
            nmu_row[bc] = nmu

            mu2 = rows.tile([1, NF], F32, tag="mu2", name=f"mu2_{bc}")
            nc.vector.tensor_mul(mu2[:], nmu[:], nmu[:])
            var = rows.tile([1, NF], F32, tag=f"var{bc}", name=f"var_{bc}")
            nc.vector.tensor_scalar_mul(var[:], psum_q[:], 1.0 / H)
            nc.vector.tensor_tensor(var[:], var[:], mu2[:],
                                    mybir.AluOpType.subtract)
            var_row[bc] = var

        def emit_stats_post(bc):
            # ACT sqrt (emitted after all phase-1 activations -> single
            # table load), then the DVE reciprocal chain
            var = var_row[bc]
            nc.scalar.activation(var[:], var[:],
                                 mybir.ActivationFunctionType.Sqrt,
                                 bias=eps_sb[:])
            rec = rows.tile([1, NF], F32, tag=f"rec{bc}", name=f"rec{bc}")
            nc.vector.reciprocal_approx_fast(rec[:], var[:])
            rrow = rows.tile([1, NF], F32R, tag=f"rstd{bc}",
                             name=f"rstd{bc}")
            nc.vector.tensor_copy(rrow[:], rec[:])
            rstd_row[bc] = rrow
            m2 = rows.tile([1, NF], F32R, tag=f"m2_{bc}", name=f"m2_{bc}")
            nc.vector.tensor_tensor(m2[:], rec[:], nmu_row[bc][:],
                                    mybir.AluOpType.mult)
            m2_row[bc] = m2

        def emit_bcast(bc):
            pb = ps.tile([P, NF], F32, tag="c0", name=f"pb{bc}")
            nc.tensor.matmul(pb[:], ones_row, rstd_row[bc][:],
                             start=True, stop=True)
            rb = rows.tile([P, NF], F32, tag=f"rstd_bc{bc}",
                           name=f"rstd_bc{bc}")
            nc.vector.tensor_copy(rb[:], pb[:])
            rstd_bc[bc] = rb
            pm = ps.tile([P, NF], F32, tag="d0", name=f"pm{bc}")
            nc.tensor.matmul(pm[:], ones_row, m2_row[bc][:],
                             start=True, stop=True)
            mb = rows.tile([P, NF], F32, tag=f"m2_bc{bc}",
                           name=f"m2_bc{bc}")
            nc.vector.tensor_copy(mb[:], pm[:])
            m2_bc[bc] = mb

        # ---- phase 1: gates + new_h --------------------------------------
        gidx = 0  # global group counter -> PSUM parity
        for ht in range(HT):
            # last ht runs bc1 first so s_acc[1] finalizes one group early
            # (its stats chain then hides under the final group + po stream)
            for bc in (range(NB) if ht < HT - 1 else (1, 0)):
                s = gidx % 2
                bs = slice(bc * NF, (bc + 1) * NF)
                pr = ps.tile([P, NF], F32, tag=TAGS[s][0],
                             name=f"pr{ht}_{bc}")
                pz = ps.tile([P, NF], F32, tag=TAGS[s][1],
                             name=f"pz{ht}_{bc}")
                pgi = ps.tile([P, NF], F32, tag=TAGS[s][2],
                              name=f"pgi{ht}_{bc}")
                pgh = ps.tile([P, NF], F32, tag=TAGS[s][3],
                              name=f"pgh{ht}_{bc}")

                for k in range(KT):
                    xs = x_sb[:, k, bs]
                    nc.tensor.matmul(pr[:], wsl(w_cur, "ir", k), xs,
                                     start=(k == 0), stop=False)
                    nc.tensor.matmul(pz[:], wsl(w_cur, "iz", k), xs,
                                     start=(k == 0), stop=False)
                    nc.tensor.matmul(pgi[:], wsl(w_cur, "in", k), xs,
                                     start=(k == 0), stop=(k == KT - 1))
                if ht == HT - 1 and bc == 0:
                    # stats(1) reduce+var mid-final-group: hides under the
                    # h-side + po stream (phase 2 runs bc1 first)
                    emit_stats_pre(1)
                for k in range(KT):
                    hs = h_sb[:, k, bs]
                    nc.tensor.matmul(pr[:], wsl(w_cur, "hr", k), hs,
                                     start=False, stop=(k == KT - 1))
                    nc.tensor.matmul(pz[:], wsl(w_cur, "hz", k), hs,
                                     start=False, stop=(k == KT - 1))
                    nc.tensor.matmul(pgh[:], wsl(w_cur, "hn", k), hs,
                                     start=(k == 0), stop=(k == KT - 1))

                # epilogue: drains under the next group's matmul stream
                r_sb = gates.tile([P, NF], F32, tag="r_act", name="r_sb")
                nc.scalar.activation(r_sb[:], pr[:],
                                     mybir.ActivationFunctionType.Sigmoid,
                                     bias=bias_col(0, ht))
                z_sb = gates.tile([P, NF], F32, tag="z_act", name="z_sb")
                nc.scalar.activation(z_sb[:], pz[:],
                                     mybir.ActivationFunctionType.Sigmoid,
                                     bias=bias_col(1, ht))

                t_sb = gates.tile([P, NF], F32, tag="t", name="t_sb")
                nc.vector.tensor_scalar(t_sb[:], pgh[:], bias_col(3, ht),
                                        None, mybir.AluOpType.add)
                nc.vector.tensor_mul(t_sb[:], t_sb[:], r_sb[:])
                nc.vector.tensor_add(t_sb[:], t_sb[:], pgi[:])
                n_sb = gates.tile([P, NF], F32, tag="n", name="n_sb")
                nc.scalar.activation(n_sb[:], t_sb[:],
                                     mybir.ActivationFunctionType.Tanh,
                                     bias=bias_col(2, ht))

                u_sb = gates.tile([P, NF], F32, tag="u", name="u_sb")
                nc.vector.tensor_tensor(u_sb[:], h_sb[:, ht, bs], n_sb[:],
                                        mybir.AluOpType.subtract)
                nc.vector.tensor_mul(u_sb[:], z_sb[:], u_sb[:])
                nhv = nh_sb[ht][:, bs]
                nc.vector.tensor_add(nhv, n_sb[:], u_sb[:])

                # LN stat partials (elementwise over h-tiles)
                if ht == 0:
                    nc.vector.tensor_copy(s_acc[bc][:], nhv)
                    nc.scalar.activation(q_acc[bc][:], nhv,
                                         mybir.ActivationFunctionType.Square)
                else:
                    sq_sb = gates.tile([P, NF], F32, tag="sq", name="sq_sb")
                    nc.vector.tensor_tensor(s_acc[bc][:],
                                            s_acc[bc][:].bitcast(F32), nhv,
                                            mybir.AluOpType.add)
                    nc.scalar.activation(sq_sb[:], nhv,
                                         mybir.ActivationFunctionType.Square)
                    nc.vector.tensor_tensor(q_acc[bc][:],
                                            q_acc[bc][:].bitcast(F32),
                                            sq_sb[:],
                                            mybir.AluOpType.add)
                gidx += 1

            nc.gpsimd.dma_start(nhT_d[ht], nh_sb[ht][:])
            if ht + 2 < HT:
                w_cur = w_nxt
                w_nxt = load_wht(ht + 2)
            elif ht + 2 == HT:
                w_cur = w_nxt
            # readout weights in two half-blocks mid-phase-1
            if ht == 2:
                nc.gpsimd.dma_start(wout_sb[:, 0:4, :, :],
                                    wout_d[:, 0:4, :, :])
            elif ht == 4:
                nc.gpsimd.dma_start(wout_sb[:, 4:OT, :, :],
                                    wout_d[:, 4:OT, :, :])

        # ---- phase 2: readout ---------------------------------------------
        # out = rstd*po - (rstd*mu)*colsum + boutF; rstd and m2=rstd*(-mu)
        # are row vectors broadcast across partitions by a ones-row matmul,
        # so the per-group finalize has no PE involvement (t1 on GpSimd, the
        # two [P,NF] combines on DVE).
        po_tags = ["a0", "b0", "a1", "b1", "c1", "d1"]
        groups = [(ot, 1) for ot in range(OT)] + [(ot, 0) for ot in range(OT)]

        t1_tiles = {}

        def emit_t1s(bc):
            # t1 = m2*colsum + boutF depends only on m2_bc: emit the whole
            # bc-batch at once so GpSimd drains early, off the tail path
            for ot in range(OT):
                t = gates.tile([P, NF], F32, tag=f"t1_{ot}",
                               name=f"t1_{ot}_{bc}")
                nc.gpsimd.tensor_scalar(t[:], m2_bc[bc][:], colsum_col(ot),
                                        bias_col(4, ot),
                                        mybir.AluOpType.mult,
                                        mybir.AluOpType.add)
                t1_tiles[(ot, bc)] = t

        def finalize(i, ot, bc, po):
            # PE-free: two DVE combines + store
            o_sb = gates.tile([P, NF], F32, tag=("r_act", "z_act", "n")[i % 3],
                              name=f"o_{ot}_{bc}")
            nc.vector.tensor_mul(o_sb[:], po[:], rstd_bc[bc][:])
            ob = gates.tile([P, NF], BF16, tag=("w1", "w2", "w3")[i % 3],
                            name=f"ob_{ot}_{bc}")
            nc.vector.tensor_tensor(ob[:], o_sb[:], t1_tiles[(ot, bc)][:],
                                    mybir.AluOpType.add)
            nc.sync.dma_start(outT_d[ot][:, bs_of(bc)], ob[:])

        def bs_of(bc):
            return slice(bc * NF, (bc + 1) * NF)

        emit_stats_post(1)
        pending = []
        for i, (ot, bc) in enumerate(groups):
            po = ps.tile([P, NF], F32, tag=po_tags[i % len(po_tags)],
                         name=f"po_{ot}_{bc}")
            for k in range(HT):
                nc.tensor.matmul(po[:], wout_sb[:, ot, k, :],
                                 nh_sb[k][:, bs_of(bc)],
                                 start=(k == 0), stop=(k == HT - 1))
            if i == 2:
                emit_stats_pre(0)
            elif i == 3:
                # bcast(1) + the bc0 sqrt chain, then flush the finalizes
                # that had to wait for rstd_bc[1]/m2_bc[1] to be enqueued
                emit_bcast(1)
                emit_stats_post(0)
                emit_t1s(1)
            elif i == 5:
                emit_bcast(0)
                emit_t1s(0)
            if i < 3:
                pending.append((i, ot, bc, po))
            else:
                for args in pending:
                    finalize(*args)
                pending = []
                finalize(i, ot, bc, po)

    nc.compile()
    return nc


def _swz_w(w):
    # [D, H] -> [HT, P, KT, P] with out[ht, p, t, c] = w[t*P+p, ht*P+c]
    return np.ascontiguousarray(
        w.reshape(KT, P, HT, P).transpose(2, 1, 0, 3)).astype(NPBF16)


def kernel(x, h, Wir, bir, Wiz, biz, Win, bin_, Whr, Whz, Whn, bhn,
           ln_scale, ln_bias, Wout, bout):
    global _COMPILED, LAST_RES
    if _COMPILED is None:
        _COMPILED = _build()
    nc = _COMPILED

    f = lambda a: np.asarray(a, np.float32)
    x, h = f(x), f(h)
    Wout, ln_scale, ln_bias = f(Wout), f(ln_scale), f(ln_bias)

    woutF = ln_scale[:, None] * Wout
    boutF = f(bout) + ln_bias @ Wout
    colsum = ln_scale @ Wout

    wall = np.empty((HT, P, len(GATES), KT, P), dtype=NPBF16)
    for gi, w in enumerate((Wir, Wiz, Win, Whr, Whz, Whn)):
        wall[:, :, gi] = _swz_w(f(w))
    wout_swz = np.ascontiguousarray(_swz_w(woutF).transpose(1, 0, 2, 3))

    bvec = np.zeros((P, 50), np.float32)
    for vi, v in enumerate((bir, biz, bin_, bhn, boutF)):
        bvec[:, 8 * vi:8 * (vi + 1)] = f(v).reshape(HT, P).T
    bvec[:, 40] = 1.0
    bvec[:, 41:49] = colsum.reshape(OT, P).T
    rowv = np.ones((1, P), np.float32)

    common = {"wall": wall, "woutF": wout_swz, "bvec": bvec, "rowv": rowv}
    in_maps = []
    for c in range(NCORES):
        bsl = slice(c * BL, (c + 1) * BL)
        xT = x[bsl].T.reshape(KT, P, BL).transpose(1, 0, 2)
        hT = h[bsl].T.reshape(KT, P, BL).transpose(1, 0, 2)
        in_maps.append({
            **common,
            "xT": np.ascontiguousarray(xT).astype(NPBF16),
            "hT": np.ascontiguousarray(hT).astype(NPBF16),
        })

    res = bass_utils.run_bass_kernel_spmd(nc, in_maps,
                                          core_ids=list(range(NCORES)),
                                          trace=TRACE)
    LAST_RES = res
    nh_parts, out_parts = [], []
    for c in range(NCORES):
        nhT = res.results[c]["nhT"].reshape(H, BL)
        outT = res.results[c]["outT"].reshape(O, BL)
        nh_parts.append(np.asarray(nhT, np.float32).T)
        out_parts.append(np.asarray(outT, np.float32).T)
    new_h = np.ascontiguousarray(np.concatenate(nh_parts, axis=0))
    out = np.ascontiguousarray(np.concatenate(out_parts, axis=0))
    return new_h, out


# revision 59
# speedup vs baseline: 1.0384x; 1.0019x over previous
"""GRUCell + LayerNorm readout fused Bass kernel for Trainium2 (8 NeuronCores).

Problem: B=8192, D=H=O=1024 fp32.
    r = sigmoid(x@Wir + bir + h@Whr)
    z = sigmoid(x@Wiz + biz + h@Whz)
    n = tanh(x@Win + bin_ + r*(h@Whn + bhn))
    new_h = (1-z)*n + z*h
    out = (LayerNorm(new_h)*ln_scale + ln_bias) @ Wout + bout

Strategy:
  - Data-parallel over batch: core c gets rows [c*1024, (c+1)*1024); weights
    replicated. No collectives.
  - Transposed domain: activations live as [feature, batch]; per-h gate
    biases become per-partition activation biases.
  - All big matmuls in bf16 (host pre-casts + pre-swizzles weights/inputs to
    the exact SBUF layout, so every DMA is 2KB-per-partition contiguous).
    Measured end-to-end error ~5.4e-3 vs the 2e-2 gate. Stats/broadcast
    matmuls stay f32r (exact fp32 width). fp8 was evaluated and rejected:
    e4m3 r/z gates alone measure 4.4e-2.
  - PSUM A/B alternation: each (ht, bc) group of 48 matmuls uses 4 banks
    (r,z,gi,gh) of one parity; its epilogue chain drains under the next
    group's 10.4us matmul stream, so the PE never waits on PSUM reuse.
  - PE warmup: 12 dummy matmuls at t=0 cover the input-DMA ramp and lift
    the HAM clock gate (1.2->2.4GHz) before real work arrives.
  - Ramp DMA: dma_start triggers cost ~0.66us of queue occupancy each, so
    the ramp-critical loads go on ONE queue (sync) in exact PE consumption
    order (x0, Wir0..Win0, x1..x3, Whr0..Whn0, h0.., x/h interleave).
    Steady-state weight tiles double-buffer one ht ahead. gpsimd queue
    carries woutF half-blocks + nhT stores; outT stores ride sync (idle in
    phase 2).
  - LayerNorm folded into the readout:
        LN(new_h) @ (ln_scale*Wout) + (ln_bias@Wout + bout)
      = rstd[b]*new_h@WoutF + m2[b]*colsum[o] + boutF[o],  m2 = -mu*rstd
    rstd/m2 row vectors are broadcast across partitions by ones-row
    matmuls, so each readout group's finalize is 3 element-wise ops spread
    over GpSimd+DVE with zero PE involvement, and the accumulation group is
    exactly the 8 contraction matmuls. LN stats accumulate elementwise over
    h-tiles, reduce across partitions via a ones-column matmul, and the
    1/sqrt uses reciprocal_approx_fast (the exact DVE reciprocal is 3.9us
    for 512 lanes on one partition and was the phase-2 critical path); the
    whole stats chain for bc1 is emitted mid-final-group so it hides under
    the last 24 gate matmuls + first readout groups (phase 2 runs bc1
    first, and the final phase-1 group order is swapped to bc1, bc0).
"""

import sys
from contextlib import ExitStack

sys.path.insert(0, "/opt/trn_rl_repo")

import numpy as np
import ml_dtypes

import concourse.bacc as bacc
import concourse.mybir as mybir
import concourse.tile as tile
from concourse import bass_utils

B, D, H, O = 8192, 1024, 1024, 1024
NCORES = 8
BL = B // NCORES          # batch rows per core
P = 128                   # partitions
KT = D // P               # contraction tiles (8)
HT = H // P               # h output-partition tiles (8)
OT = O // P               # o output-partition tiles (8)
NB = 2                    # batch chunks per core
NF = BL // NB             # free dim per chunk (512)
LN_EPS = 1e-6
NWARM = 12                # PE warmup matmuls

F32 = mybir.dt.float32
F32R = mybir.dt.float32r
BF16 = mybir.dt.bfloat16
NPBF16 = ml_dtypes.bfloat16

_COMPILED = None
TRACE = False
LAST_RES = None

GATES = ("ir", "iz", "in", "hr", "hz", "hn")


def _build():
    nc = bacc.Bacc("TRN2", target_bir_lowering=False, debug=False,
                   num_devices=NCORES)

    # all per-partition-contiguous layouts so DMA descriptors are 2-16KB
    xT_d = nc.dram_tensor("xT", [P, KT, BL], BF16, kind="ExternalInput").ap()
    hT_d = nc.dram_tensor("hT", [P, KT, BL], BF16, kind="ExternalInput").ap()
    wall_d = nc.dram_tensor("wall", [HT, P, len(GATES), KT, P], BF16,
                            kind="ExternalInput").ap()
    wout_d = nc.dram_tensor("woutF", [P, OT, KT, P], BF16,
                            kind="ExternalInput").ap()
    # [P, 50]: bir | biz | bin | bhn | boutF (8 cols each, col t = tile t),
    # col 40 = ones (stats-reduce stationary), cols 41:49 = colsum tiles
    bvec_d = nc.dram_tensor("bvec", [P, 50], F32R, kind="ExternalInput").ap()
    # [1, P]: ones_row (broadcast stationary)
    rowv_d = nc.dram_tensor("rowv", [1, P], F32R,
                            kind="ExternalInput").ap()

    nhT_d = nc.dram_tensor("nhT", [HT, P, BL], BF16,
                           kind="ExternalOutput").ap()
    outT_d = nc.dram_tensor("outT", [OT, P, BL], BF16,
                            kind="ExternalOutput").ap()

    with tile.TileContext(nc) as tc, ExitStack() as ctx:
        singles = ctx.enter_context(tc.tile_pool(name="singles", bufs=1))
        wpool = ctx.enter_context(tc.tile_pool(name="wpool", bufs=2))
        gates = ctx.enter_context(tc.tile_pool(name="gates", bufs=1))
        rows = ctx.enter_context(tc.tile_pool(name="rows", bufs=1))
        ps = ctx.enter_context(tc.tile_pool(name="ps", bufs=1, space="PSUM"))

        TAGS = [["a0", "b0", "c0", "d0"], ["a1", "b1", "c1", "d1"]]

        # ---- PE warmup: dummy matmuls cover the DMA ramp, lift HAM -------
        wm = singles.tile([P, 2 * P], BF16, tag="wm", name="wm")
        nc.vector.memset(wm[:], 0.0)
        for i in range(NWARM):
            pw = ps.tile([P, P], F32, tag="a1", name=f"warm{i}")
            nc.tensor.matmul(pw[:], wm[:, 0:P], wm[:, P:2 * P],
                             start=True, stop=True)

        # ---- resident inputs --------------------------------------------
        x_sb = singles.tile([P, KT, BL], BF16, tag="x_sb", name="x_sb")
        h_sb = singles.tile([P, KT, BL], BF16, tag="h_sb", name="h_sb")

        def wtile(g, ht):
            t = wpool.tile([P, KT, P], BF16, tag=f"w{g}", name=f"w{g}_{ht}")
            nc.sync.dma_start(t[:], wall_d[ht][:, GATES.index(g), :, :])
            return t

        def load_wht(ht):
            return {g: wtile(g, ht) for g in GATES}

        def wsl(w, g, k):
            return w[g][:, k, :]

        # ramp: ONE queue (per-queue FIFO) in exact PE consumption order,
        # per-k slices so arrival granularity matches consumption
        w_cur = {}
        nc.sync.dma_start(x_sb[:, 0, :], xT_d[:, 0, :])
        for g in ("ir", "iz", "in"):
            w_cur[g] = wtile(g, 0)
        for k in (1, 2, 3):
            nc.sync.dma_start(x_sb[:, k, :], xT_d[:, k, :])
        for g in ("hr", "hz", "hn"):
            w_cur[g] = wtile(g, 0)
        for k in (0, 1):
            nc.sync.dma_start(h_sb[:, k, :], hT_d[:, k, :])
        for xk, hk in ((4, 2), (5, 3), (6, 4), (7, 5)):
            nc.sync.dma_start(x_sb[:, xk, :], xT_d[:, xk, :])
            nc.sync.dma_start(h_sb[:, hk, :], hT_d[:, hk, :])
        for k in (6, 7):
            nc.sync.dma_start(h_sb[:, k, :], hT_d[:, k, :])
        w_nxt = load_wht(1)

        bvec = singles.tile([P, 50], F32R, tag="bvec", name="bvec")
        nc.scalar.dma_start(bvec[:], bvec_d)
        rowv = singles.tile([1, P], F32R, tag="rowv", name="rowv")
        nc.scalar.dma_start(rowv[:], rowv_d)
        eps_sb = singles.tile([1, 1], F32, tag="eps", name="eps")
        nc.vector.memset(eps_sb[:], LN_EPS)
        ones_row_bf = singles.tile([1, P], BF16, tag="ones_bf",
                                   name="ones_bf")
        nc.vector.memset(ones_row_bf[:], 1.0)

        def bias_col(v, t):  # v: 0=bir 1=biz 2=bin 3=bhn 4=boutF
            return bvec[:, 8 * v + t:8 * v + t + 1].bitcast(F32)

        ones_col = bvec[:, 40:41]
        ones_row = rowv[:, 0:P]

        def colsum_col(ot):  # [P,1] per-partition scalar, f32 view
            return bvec[:, 41 + ot:42 + ot].bitcast(F32)

        wout_sb = singles.tile([P, OT, KT, P], BF16, tag="wout",
                               name="wout_sb")

        nh_sb = [singles.tile([P, BL], BF16, tag=f"nh{ht}", name=f"nh{ht}")
                 for ht in range(HT)]
        s_acc = [singles.tile([P, NF], F32R, tag=f"s_acc{bc}",
                              name=f"s_acc{bc}") for bc in range(NB)]
        q_acc = [singles.tile([P, NF], F32R, tag=f"q_acc{bc}",
                              name=f"q_acc{bc}") for bc in range(NB)]

        # ---- LN stats helpers (emitted inside/right after phase 1 so the
        # ---- 3.9us DVE RECIPROCAL hides under matmul cover) ---------------
        rstd_row = {}
        m2_row = {}
        rstd_bc = {}
        m2_bc = {}

        nmu_row = {}
        var_row = {}

        def emit_stats_pre(bc):
            # PE reduce matmuls + DVE-only var chain (no ACT involvement,
            # so the group-15 sigmoid/tanh sequence never waits on an ACT
            # table reload for Sqrt)
            psum_s = ps.tile([1, NF], F32, tag="c0", name=f"psum_s{bc}")
            nc.tensor.matmul(psum_s[:], ones_col, s_acc[bc][:],
                             start=True, stop=True)
            psum_q = ps.tile([1, NF], F32, tag="d0", name=f"psum_q{bc}")
            nc.tensor.matmul(psum_q[:], ones_col, q_acc[bc][:],
                             start=True, stop=True)

            nmu = rows.tile([1, NF], F32, tag=f"nmu{bc}", name=f"nmu{bc}")
            nc.vector.tensor_scalar_mul(nmu[:], psum_s[:], -1.0 / H)
            nmu_row[bc] = nmu

            mu2 = rows.tile([1, NF], F32, tag="mu2", name=f"mu2_{bc}")
            nc.vector.tensor_mul(mu2[:], nmu[:], nmu[:])
            var = rows.tile([1, NF], F32, tag=f"var{bc}", name=f"var_{bc}")
            nc.vector.tensor_scalar_mul(var[:], psum_q[:], 1.0 / H)
            nc.vector.tensor_tensor(var[:], var[:], mu2[:],
                                    mybir.AluOpType.subtract)
            var_row[bc] = var

        def emit_stats_post(bc):
            # ACT sqrt (emitted after all phase-1 activations -> single
            # table load), then the DVE reciprocal chain
            var = var_row[bc]
            nc.scalar.activation(var[:], var[:],
                                 mybir.ActivationFunctionType.Sqrt,
                                 bias=eps_sb[:])
            rec = rows.tile([1, NF], F32, tag=f"rec{bc}", name=f"rec{bc}")
            nc.vector.reciprocal_approx_fast(rec[:], var[:])
            # bf16 broadcast rows: the K=1 f32r matmul runs 2 cyc/row
            # (426ns); bf16 runs 216ns. Rounding cost measured 5.1e-3 on
            # out (vs 5.4e-3 already) -- negligible
            rrow = rows.tile([1, NF], BF16, tag=f"rstd{bc}",
                             name=f"rstd{bc}")
            nc.vector.tensor_copy(rrow[:], rec[:])
            rstd_row[bc] = rrow
            m2 = rows.tile([1, NF], BF16, tag=f"m2_{bc}", name=f"m2_{bc}")
            nc.vector.tensor_tensor(m2[:], rec[:], nmu_row[bc][:],
                                    mybir.AluOpType.mult)
            m2_row[bc] = m2

        def emit_bcast(bc):
            pb = ps.tile([P, NF], F32, tag="c0", name=f"pb{bc}")
            nc.tensor.matmul(pb[:], ones_row_bf[:], rstd_row[bc][:],
                             start=True, stop=True)
            rb = rows.tile([P, NF], F32, tag=f"rstd_bc{bc}",
                           name=f"rstd_bc{bc}")
            nc.vector.tensor_copy(rb[:], pb[:])
            rstd_bc[bc] = rb
            pm = ps.tile([P, NF], F32, tag="d0", name=f"pm{bc}")
            nc.tensor.matmul(pm[:], ones_row_bf[:], m2_row[bc][:],
                             start=True, stop=True)
            mb = rows.tile([P, NF], F32, tag=f"m2_bc{bc}",
                           name=f"m2_bc{bc}")
            nc.vector.tensor_copy(mb[:], pm[:])
            m2_bc[bc] = mb

        # ---- phase 1: gates + new_h --------------------------------------
        gidx = 0  # global group counter -> PSUM parity
        for ht in range(HT):
            # last ht runs bc1 first so s_acc[1] finalizes one group early
            # (its stats chain then hides under the final group + po stream)
            for bc in (range(NB) if ht < HT - 1 else (1, 0)):
                s = gidx % 2
                bs = slice(bc * NF, (bc + 1) * NF)
                pr = ps.tile([P, NF], F32, tag=TAGS[s][0],
                             name=f"pr{ht}_{bc}")
                pz = ps.tile([P, NF], F32, tag=TAGS[s][1],
                             name=f"pz{ht}_{bc}")
                pgi = ps.tile([P, NF], F32, tag=TAGS[s][2],
                              name=f"pgi{ht}_{bc}")
                pgh = ps.tile([P, NF], F32, tag=TAGS[s][3],
                              name=f"pgh{ht}_{bc}")

                for k in range(KT):
                    xs = x_sb[:, k, bs]
                    nc.tensor.matmul(pr[:], wsl(w_cur, "ir", k), xs,
                                     start=(k == 0), stop=False)
                    nc.tensor.matmul(pz[:], wsl(w_cur, "iz", k), xs,
                                     start=(k == 0), stop=False)
                    nc.tensor.matmul(pgi[:], wsl(w_cur, "in", k), xs,
                                     start=(k == 0), stop=(k == KT - 1))
                if ht == HT - 1 and bc == 0:
                    # stats(1) reduce+var mid-final-group: hides under the
                    # h-side + po stream (phase 2 runs bc1 first)
                    emit_stats_pre(1)
                for k in range(KT):
                    hs = h_sb[:, k, bs]
                    nc.tensor.matmul(pr[:], wsl(w_cur, "hr", k), hs,
                                     start=False, stop=(k == KT - 1))
                    nc.tensor.matmul(pz[:], wsl(w_cur, "hz", k), hs,
                                     start=False, stop=(k == KT - 1))
                    nc.tensor.matmul(pgh[:], wsl(w_cur, "hn", k), hs,
                                     start=(k == 0), stop=(k == KT - 1))

                # epilogue: drains under the next group's matmul stream
                r_sb = gates.tile([P, NF], F32, tag="r_act", name="r_sb")
                nc.scalar.activation(r_sb[:], pr[:],
                                     mybir.ActivationFunctionType.Sigmoid,
                                     bias=bias_col(0, ht))
                z_sb = gates.tile([P, NF], F32, tag="z_act", name="z_sb")
                nc.scalar.activation(z_sb[:], pz[:],
                                     mybir.ActivationFunctionType.Sigmoid,
                                     bias=bias_col(1, ht))

                t_sb = gates.tile([P, NF], F32, tag="t", name="t_sb")
                nc.vector.tensor_scalar(t_sb[:], pgh[:], bias_col(3, ht),
                                        None, mybir.AluOpType.add)
                nc.vector.tensor_mul(t_sb[:], t_sb[:], r_sb[:])
                nc.vector.tensor_add(t_sb[:], t_sb[:], pgi[:])
                n_sb = gates.tile([P, NF], F32, tag="n", name="n_sb")
                nc.scalar.activation(n_sb[:], t_sb[:],
                                     mybir.ActivationFunctionType.Tanh,
                                     bias=bias_col(2, ht))

                u_sb = gates.tile([P, NF], F32, tag="u", name="u_sb")
                nc.vector.tensor_tensor(u_sb[:], h_sb[:, ht, bs], n_sb[:],
                                        mybir.AluOpType.subtract)
                nc.vector.tensor_mul(u_sb[:], z_sb[:], u_sb[:])
                nhv = nh_sb[ht][:, bs]
                nc.vector.tensor_add(nhv, n_sb[:], u_sb[:])

                # LN stat partials (elementwise over h-tiles)
                if ht == 0:
                    nc.vector.tensor_copy(s_acc[bc][:], nhv)
                    nc.scalar.activation(q_acc[bc][:], nhv,
                                         mybir.ActivationFunctionType.Square)
                else:
                    sq_sb = gates.tile([P, NF], F32, tag="sq", name="sq_sb")
                    nc.vector.tensor_tensor(s_acc[bc][:],
                                            s_acc[bc][:].bitcast(F32), nhv,
                                            mybir.AluOpType.add)
                    nc.scalar.activation(sq_sb[:], nhv,
                                         mybir.ActivationFunctionType.Square)
                    nc.vector.tensor_tensor(q_acc[bc][:],
                                            q_acc[bc][:].bitcast(F32),
                                            sq_sb[:],
                                            mybir.AluOpType.add)
                gidx += 1

            nc.gpsimd.dma_start(nhT_d[ht], nh_sb[ht][:])
            if ht + 2 < HT:
                w_cur = w_nxt
                w_nxt = load_wht(ht + 2)
            elif ht + 2 == HT:
                w_cur = w_nxt
            # readout weights in two half-blocks mid-phase-1
            if ht == 2:
                nc.gpsimd.dma_start(wout_sb[:, 0:4, :, :],
                                    wout_d[:, 0:4, :, :])
            elif ht == 4:
                nc.gpsimd.dma_start(wout_sb[:, 4:OT, :, :],
                                    wout_d[:, 4:OT, :, :])

        # ---- phase 2: readout ---------------------------------------------
        # out = rstd*po - (rstd*mu)*colsum + boutF; rstd and m2=rstd*(-mu)
        # are row vectors broadcast across partitions by a ones-row matmul,
        # so the per-group finalize has no PE involvement (t1 on GpSimd, the
        # two [P,NF] combines on DVE).
        po_tags = ["a0", "b0", "a1", "b1", "c1", "d1"]
        groups = [(ot, 1) for ot in range(OT)] + [(ot, 0) for ot in range(OT)]

        t1_tiles = {}

        def emit_t1s(bc):
            # t1 = m2*colsum + boutF depends only on m2_bc: emit the whole
            # bc-batch at once so GpSimd drains early, off the tail path
            for ot in range(OT):
                t = gates.tile([P, NF], F32, tag=f"t1_{ot}",
                               name=f"t1_{ot}_{bc}")
                nc.gpsimd.tensor_scalar(t[:], m2_bc[bc][:], colsum_col(ot),
                                        bias_col(4, ot),
                                        mybir.AluOpType.mult,
                                        mybir.AluOpType.add)
                t1_tiles[(ot, bc)] = t

        def finalize(i, ot, bc, po):
            # PE-free: two DVE combines + store
            o_sb = gates.tile([P, NF], F32, tag=("r_act", "z_act", "n")[i % 3],
                              name=f"o_{ot}_{bc}")
            nc.vector.tensor_mul(o_sb[:], po[:], rstd_bc[bc][:])
            ob = gates.tile([P, NF], BF16, tag=("w1", "w2", "w3")[i % 3],
                            name=f"ob_{ot}_{bc}")
            nc.vector.tensor_tensor(ob[:], o_sb[:], t1_tiles[(ot, bc)][:],
                                    mybir.AluOpType.add)
            nc.sync.dma_start(outT_d[ot][:, bs_of(bc)], ob[:])

        def bs_of(bc):
            return slice(bc * NF, (bc + 1) * NF)

        emit_stats_post(1)
        pending = []
        for i, (ot, bc) in enumerate(groups):
            po = ps.tile([P, NF], F32, tag=po_tags[i % len(po_tags)],
                         name=f"po_{ot}_{bc}")
            for k in range(HT):
                nc.tensor.matmul(po[:], wout_sb[:, ot, k, :],
                                 nh_sb[k][:, bs_of(bc)],
                                 start=(k == 0), stop=(k == HT - 1))
            if i == 2:
                emit_stats_pre(0)
            elif i == 3:
                # bcast(1) + the bc0 sqrt chain, then flush the finalizes
                # that had to wait for rstd_bc[1]/m2_bc[1] to be enqueued
                emit_bcast(1)
                emit_stats_post(0)
                emit_t1s(1)
            elif i == 5:
                emit_bcast(0)
                emit_t1s(0)
            if i < 3:
                pending.append((i, ot, bc, po))
            else:
                for args in pending:
                    finalize(*args)
                pending = []
                finalize(i, ot, bc, po)

    nc.compile()
    return nc


def _swz_w(w):
    # [D, H] -> [HT, P, KT, P] with out[ht, p, t, c] = w[t*P+p, ht*P+c]
    return np.ascontiguousarray(
        w.reshape(KT, P, HT, P).transpose(2, 1, 0, 3)).astype(NPBF16)


def kernel(x, h, Wir, bir, Wiz, biz, Win, bin_, Whr, Whz, Whn, bhn,
           ln_scale, ln_bias, Wout, bout):
    global _COMPILED, LAST_RES
    if _COMPILED is None:
        _COMPILED = _build()
    nc = _COMPILED

    f = lambda a: np.asarray(a, np.float32)
    x, h = f(x), f(h)
    Wout, ln_scale, ln_bias = f(Wout), f(ln_scale), f(ln_bias)

    woutF = ln_scale[:, None] * Wout
    boutF = f(bout) + ln_bias @ Wout
    colsum = ln_scale @ Wout

    wall = np.empty((HT, P, len(GATES), KT, P), dtype=NPBF16)
    for gi, w in enumerate((Wir, Wiz, Win, Whr, Whz, Whn)):
        wall[:, :, gi] = _swz_w(f(w))
    wout_swz = np.ascontiguousarray(_swz_w(woutF).transpose(1, 0, 2, 3))

    bvec = np.zeros((P, 50), np.float32)
    for vi, v in enumerate((bir, biz, bin_, bhn, boutF)):
        bvec[:, 8 * vi:8 * (vi + 1)] = f(v).reshape(HT, P).T
    bvec[:, 40] = 1.0
    bvec[:, 41:49] = colsum.reshape(OT, P).T
    rowv = np.ones((1, P), np.float32)

    common = {"wall": wall, "woutF": wout_swz, "bvec": bvec, "rowv": rowv}
    in_maps = []
    for c in range(NCORES):
        bsl = slice(c * BL, (c + 1) * BL)
        xT = x[bsl].T.reshape(KT, P, BL).transpose(1, 0, 2)
        hT = h[bsl].T.reshape(KT, P, BL).transpose(1, 0, 2)
        in_maps.append({
            **common,
            "xT": np.ascontiguousarray(xT).astype(NPBF16),
            "hT": np.ascontiguousarray(hT).astype(NPBF16),
        })

    res = bass_utils.run_bass_kernel_spmd(nc, in_maps,
                                          core_ids=list(range(NCORES)),
                                          trace=TRACE)
    LAST_RES = res
    nh_parts, out_parts = [], []
    for c in range(NCORES):
        nhT = res.results[c]["nhT"].reshape(H, BL)
        outT = res.results[c]["outT"].reshape(O, BL)
        nh_parts.append(np.asarray(nhT, np.float32).T)
        out_parts.append(np.asarray(outT, np.float32).T)
    new_h = np.ascontiguousarray(np.concatenate(nh_parts, axis=0))
    out = np.ascontiguousarray(np.concatenate(out_parts, axis=0))
    return new_h, out
